# revision 38
# baseline (speedup 1.0000x reference)
"""DualRelGCN message-passing kernel for 8 TRN2 NeuronCores.

Strategy (destination-sharded, collective-free, block-dense):
  - LayerNorm is invariant to positive per-row scaling, so LN(agg/denom) ==
    LN(agg): the denominator drops out of the computation entirely.
  - Shard edges by dst range: core c owns nodes [1250c, 1250(c+1)) and
    receives every edge whose dst falls there.  Each core computes its 1250
    output rows completely locally -> no collectives.
  - The weighted gather+segment_sum is expressed as a block matmul:
    agg[tile t] = sum_s W_ts.T @ X_s, where W_ts is the [128 src, 128 dst]
    dense block of the weighted adjacency (host-scattered from the edge
    list; ~5% nnz but dense matmul on PE beats any descriptor-generated
    gather path by a wide margin) and X_s is a [128, 256] tile of rel_embed
    (bf16, fully resident in SBUF).  PSUM accumulates over s in fp32.
  - Epilogue per dst tile: LN on DVE/ACT, PE transpose, y = ln @ proj_w.T,
    out = rel_embed + 0.1*y.
  - The device program is fully static: the edge distribution only changes
    tensor *contents*, never the instruction stream.
"""

import sys

for _p in ("/opt/trn_rl_repo",):
    if _p not in sys.path:
        sys.path.insert(0, _p)

from contextlib import ExitStack

import numpy as np
import ml_dtypes

import concourse.bacc as bacc
import concourse.mybir as mybir
from concourse.alu_op_type import AluOpType
from concourse.tile import TileContext
from concourse.bass_utils import run_bass_kernel_spmd

F32 = mybir.dt.float32
BF16 = mybir.dt.bfloat16
FP8 = mybir.dt.float8e4
AF = mybir.ActivationFunctionType

N_NODES = 10000
DIM = 256
N_CORES = 8
NODES_PER_CORE = N_NODES // N_CORES  # 1250
TILE = 128
N_TILES = -(-NODES_PER_CORE // TILE)  # 10 dst tiles per core
S_TILES = -(-N_NODES // TILE)  # 79 src tiles
OUT_ROWS = N_TILES * TILE  # 1280
ALPHA = 0.1
LN_EPS = 1e-5
# chunk boundaries (src-tile units): small leading chunks let the first
# matmuls start before the bulk of X/W lands
X_BOUNDS = [0, 20, 40, 60, 79]
XDIM = DIM + 1  # X tiles carry sum_f(x) as column 256 -> rowsum lands in PSUM
W_BOUNDS0 = [0, 20, 40, 60, 79]  # dst tile 0 (startup critical)
W_BOUNDS = [0, 20, 40, 60, 79]  # steady-state tiles

_CACHE: dict = {}


def _build():
    nc = bacc.Bacc("TRN2", target_bir_lowering=False, debug=False,
                   num_devices=N_CORES)

    x_d = nc.dram_tensor("x", [128, S_TILES * XDIM], FP8,
                        kind="ExternalInput")
    w_d = nc.dram_tensor("wblk", [N_TILES, 128, S_TILES * TILE], FP8,
                         kind="ExternalInput")
    relsl_d = nc.dram_tensor("relslice", [OUT_ROWS, DIM], F32,
                             kind="ExternalInput")
    pwt_d = nc.dram_tensor("projwT", [128, 2 * DIM], BF16,
                           kind="ExternalInput")
    out_d = nc.dram_tensor("out", [OUT_ROWS, DIM], F32, kind="ExternalOutput")

    with TileContext(nc) as tc, ExitStack() as es:
        const_pool = es.enter_context(tc.tile_pool(name="const", bufs=1))
        wpool = es.enter_context(tc.tile_pool(name="wblk", bufs=2))
        ep_pool = es.enter_context(tc.tile_pool(name="ep", bufs=2))
        ps_agg = es.enter_context(tc.tile_pool(name="ps_agg", bufs=4,
                                               space="PSUM"))
        ps_tr = es.enter_context(tc.tile_pool(name="ps_tr", bufs=2,
                                              space="PSUM"))
        ps_y = es.enter_context(tc.tile_pool(name="ps_y", bufs=2,
                                             space="PSUM"))

        # --- constants / resident inputs ---
        iota_row = const_pool.tile([128, 128], F32, tag="iota")
        nc.gpsimd.iota(iota_row[:], [[1, 128]], base=0, channel_multiplier=0,
                       allow_small_or_imprecise_dtypes=True)
        pidx = const_pool.tile([128, 1], F32, tag="pidx")
        nc.gpsimd.iota(pidx[:], [[1, 1]], base=0, channel_multiplier=1,
                       allow_small_or_imprecise_dtypes=True)
        ident = const_pool.tile([128, 128], BF16, tag="ident")
        nc.vector.tensor_scalar(ident[:], iota_row[:], pidx[:], None,
                                AluOpType.is_equal)
        epsb = const_pool.tile([128, 1], F32, tag="epsb")
        nc.vector.memset(epsb[:], LN_EPS)
        pwt_sb = const_pool.tile([128, 2, DIM], BF16, tag="pwt")
        nc.scalar.dma_start(pwt_sb[:], pwt_d[:])  # scalar: off W's ring

        # rel_embed (bf16), fully resident; chunked load so dst-tile 0's
        # matmuls can start before the whole 5 MB lands.  X rides the
        # scalar-engine HWDGE ring so it doesn't queue ahead of W's
        # sync-engine ring (per-engine FIFO).
        x_sb = const_pool.tile([128, S_TILES, XDIM], FP8, tag="x")
        bounds = X_BOUNDS
        for i in range(len(bounds) - 1):
            lo, hi = bounds[i], bounds[i + 1]
            nc.scalar.dma_start(x_sb[:, lo:hi, :],
                                x_d[:, lo * XDIM:hi * XDIM])

        def epilogue(t, agg_ps):
            # LN -> transpose -> @ proj_w.T -> residual.  PSUM col 256 holds
            # rowsum (X's 257th column trick); centered values are computed
            # straight from PSUM, so no agg copy sits on the critical chain.
            mean = ep_pool.tile([128, 1], F32, tag="mean")
            nc.scalar.mul(mean[:], agg_ps[:, 256:257], 1.0 / DIM)
            cent = ep_pool.tile([128, DIM], F32, tag="cent")
            nc.vector.tensor_scalar(cent[:], agg_ps[:, 0:DIM], mean[:], None,
                                    AluOpType.subtract)
            sq = ep_pool.tile([128, DIM], F32, tag="sq")
            sumsq = ep_pool.tile([128, 1], F32, tag="sumsq")
            nc.scalar.activation(sq[:], cent[:], AF.Square,
                                 accum_out=sumsq[:])
            std = ep_pool.tile([128, 1], F32, tag="std")
            nc.scalar.activation(std[:], sumsq[:], AF.Sqrt, bias=epsb[:],
                                 scale=1.0 / DIM)
            rstd = ep_pool.tile([128, 1], F32, tag="rstd")
            nc.vector.reciprocal(rstd[:], std[:])
            ln = ep_pool.tile([128, DIM], BF16, tag="ln")
            nc.vector.tensor_scalar(ln[:], cent[:], rstd[:], None,
                                    AluOpType.mult)

            y_ps = ps_y.tile([128, DIM], F32, tag="y")
            for k in range(2):
                tr_ps = ps_tr.tile([128, 128], BF16, tag="tr")
                nc.tensor.transpose(tr_ps[:], ln[:, k * 128:(k + 1) * 128],
                                    ident[:])
                lnT = ep_pool.tile([128, 128], BF16, tag="lnT")
                nc.scalar.copy(lnT[:], tr_ps[:])
                nc.tensor.matmul(y_ps[:], lnT[:], pwt_sb[:, k, :],
                                 start=(k == 0), stop=(k == 1))

            rel_t = ep_pool.tile([128, DIM], F32, tag="rel")
            nc.scalar.dma_start(rel_t[:], relsl_d[t * 128:(t + 1) * 128, :])
            delta = ep_pool.tile([128, DIM], F32, tag="delta")
            nc.vector.tensor_scalar(delta[:], y_ps[:], ALPHA, None,
                                    AluOpType.mult)
            out_t = ep_pool.tile([128, DIM], F32, tag="out")
            nc.vector.tensor_tensor(out_t[:], delta[:], rel_t[:],
                                    AluOpType.add)
            nc.sync.dma_start(out_d[t * 128:(t + 1) * 128, :], out_t[:])

        # software-pipelined: tile t's block MMs are emitted before tile
        # t-1's epilogue, so the PE (strict program order) never stalls on
        # the previous tile's LN chain -- it hides under the next MM run
        pending = []
        for t in range(N_TILES):
            wb = W_BOUNDS0 if t == 0 else W_BOUNDS
            w_t = wpool.tile([128, S_TILES, TILE], FP8, tag="w")
            for i in range(len(wb) - 1):
                lo, hi = wb[i], wb[i + 1]
                nc.sync.dma_start(w_t[:, lo:hi, :],
                                  w_d[t, :, lo * TILE:hi * TILE])

            agg_ps = ps_agg.tile([128, XDIM], F32, tag="agg")
            for s in range(S_TILES):
                nc.tensor.matmul(agg_ps[:], w_t[:, s, :], x_sb[:, s, :],
                                 start=(s == 0), stop=(s == S_TILES - 1))
            pending.append((t, agg_ps))
            if len(pending) > 1:  # depth-2: epilogue(t-1) after MMs(t)... wait
                epilogue(*pending.pop(0))
        for p in pending:
            epilogue(*p)

    nc.compile()
    return nc


def _prep(rel_embed, rel_edge_index, rel_edge_weight, proj_w):
    """Host-side sharding/layout: scatter edges into dense per-(dst tile,
    src tile) weight blocks; lay out rel_embed for SBUF residency."""
    src = np.asarray(rel_edge_index[0], dtype=np.int64)
    dst = np.asarray(rel_edge_index[1], dtype=np.int64)
    w = np.asarray(rel_edge_weight, dtype=np.float32)
    rel = np.asarray(rel_embed, dtype=np.float32)
    pw = np.asarray(proj_w, dtype=np.float32)

    core = dst // NODES_PER_CORE
    drel = dst - core * NODES_PER_CORE
    t = drel // TILE
    d = drel % TILE
    s = src // TILE
    p = src % TILE
    # flat index inside one core's [N_TILES, S_TILES, 128, 128] block array
    flat = ((t * S_TILES + s) * TILE + p) * TILE + d
    blk_sz = N_TILES * S_TILES * TILE * TILE

    w_dev = np.empty((N_CORES, N_TILES, 128, S_TILES * TILE),
                     dtype=ml_dtypes.float8_e4m3)
    for c in range(N_CORES):
        m = core == c
        wc = np.bincount(flat[m], weights=w[m], minlength=blk_sz)
        wc = wc.reshape(N_TILES, S_TILES, TILE, TILE).astype(np.float32)
        # -> [t, p(src), s*128+d(dst)] so the SBUF tile is partition=src
        w_dev[c] = wc.transpose(0, 2, 1, 3).reshape(
            N_TILES, 128, S_TILES * TILE)

    rel16 = rel.astype(ml_dtypes.float8_e4m3)
    rel16_pad = np.zeros((S_TILES * TILE, XDIM), dtype=ml_dtypes.float8_e4m3)
    rel16_pad[:N_NODES, :DIM] = rel16
    rel16_pad[:N_NODES, DIM] = rel16.astype(np.float32).sum(axis=1).astype(
        ml_dtypes.float8_e4m3)
    x_dev = np.ascontiguousarray(
        rel16_pad.reshape(S_TILES, TILE, XDIM).transpose(1, 0, 2).reshape(
            128, S_TILES * XDIM))

    relslice = np.zeros((N_CORES, OUT_ROWS, DIM), dtype=np.float32)
    for c in range(N_CORES):
        relslice[c, :NODES_PER_CORE] = rel[c * NODES_PER_CORE:
                                           (c + 1) * NODES_PER_CORE]
    pwt = pw.T.astype(ml_dtypes.bfloat16)  # [f, o]
    pwt_dev = np.ascontiguousarray(
        pwt.reshape(2, 128, DIM).transpose(1, 0, 2).reshape(128, 2 * DIM))

    in_maps = []
    for c in range(N_CORES):
        in_maps.append({
            "x": x_dev,
            "wblk": w_dev[c],
            "relslice": relslice[c],
            "projwT": pwt_dev,
        })
    return in_maps


def kernel(rel_embed, rel_edge_index, rel_edge_weight, proj_w,
           _trace=False):
    in_maps = _prep(rel_embed, rel_edge_index, rel_edge_weight, proj_w)
    nc = _CACHE.get("nc")
    if nc is None:
        nc = _build()
        _CACHE["nc"] = nc
    res = run_bass_kernel_spmd(nc, in_maps, core_ids=list(range(N_CORES)),
                               trace=_trace)
    out = np.concatenate(
        [res.results[c]["out"][:NODES_PER_CORE] for c in range(N_CORES)],
        axis=0)
    if _trace:
        kernel.last_results = res
    return out.astype(np.float32)


# revision 39
# speedup vs baseline: 1.0056x; 1.0056x over previous
"""DualRelGCN message-passing kernel for 8 TRN2 NeuronCores.

Strategy (destination-sharded, collective-free, block-dense):
  - LayerNorm is invariant to positive per-row scaling, so LN(agg/denom) ==
    LN(agg): the denominator drops out of the computation entirely.
  - Shard edges by dst range: core c owns nodes [1250c, 1250(c+1)) and
    receives every edge whose dst falls there.  Each core computes its 1250
    output rows completely locally -> no collectives.
  - The weighted gather+segment_sum is expressed as a block matmul:
    agg[tile t] = sum_s W_ts.T @ X_s, where W_ts is the [128 src, 128 dst]
    dense block of the weighted adjacency (host-scattered from the edge
    list; ~5% nnz but dense matmul on PE beats any descriptor-generated
    gather path by a wide margin) and X_s is a [128, 256] tile of rel_embed
    (bf16, fully resident in SBUF).  PSUM accumulates over s in fp32.
  - Epilogue per dst tile: LN on DVE/ACT, PE transpose, y = ln @ proj_w.T,
    out = rel_embed + 0.1*y.
  - The device program is fully static: the edge distribution only changes
    tensor *contents*, never the instruction stream.
"""

import sys

for _p in ("/opt/trn_rl_repo",):
    if _p not in sys.path:
        sys.path.insert(0, _p)

from contextlib import ExitStack

import numpy as np
import ml_dtypes

import concourse.bacc as bacc
import concourse.mybir as mybir
from concourse.alu_op_type import AluOpType
from concourse.tile import TileContext
from concourse.bass_utils import run_bass_kernel_spmd

F32 = mybir.dt.float32
BF16 = mybir.dt.bfloat16
FP8 = mybir.dt.float8e4
AF = mybir.ActivationFunctionType

N_NODES = 10000
DIM = 256
N_CORES = 8
NODES_PER_CORE = N_NODES // N_CORES  # 1250
TILE = 128
N_TILES = -(-NODES_PER_CORE // TILE)  # 10 dst tiles per core
S_TILES = -(-N_NODES // TILE)  # 79 src tiles
OUT_ROWS = N_TILES * TILE  # 1280
ALPHA = 0.1
LN_EPS = 1e-5
# chunk boundaries (src-tile units): small leading chunks let the first
# matmuls start before the bulk of X/W lands
X_BOUNDS = [0, 20, 40, 60, 79]
XDIM = DIM + 1  # X tiles carry sum_f(x) as column 256 -> rowsum lands in PSUM
W_BOUNDS0 = [0, 20, 40, 60, 79]  # dst tile 0 (startup critical)
W_BOUNDS = [0, 20, 40, 60, 79]  # steady-state tiles

_CACHE: dict = {}


def _build():
    nc = bacc.Bacc("TRN2", target_bir_lowering=False, debug=False,
                   num_devices=N_CORES)

    x_d = nc.dram_tensor("x", [128, S_TILES * XDIM], FP8,
                        kind="ExternalInput")
    w_d = nc.dram_tensor("wblk", [N_TILES, 128, S_TILES * TILE], FP8,
                         kind="ExternalInput")
    relsl_d = nc.dram_tensor("relslice", [OUT_ROWS, DIM], F32,
                             kind="ExternalInput")
    pwt_d = nc.dram_tensor("projwT", [128, 2 * DIM], BF16,
                           kind="ExternalInput")
    out_d = nc.dram_tensor("out", [OUT_ROWS, DIM], F32, kind="ExternalOutput")

    with TileContext(nc) as tc, ExitStack() as es:
        const_pool = es.enter_context(tc.tile_pool(name="const", bufs=1))
        wpool = es.enter_context(tc.tile_pool(name="wblk", bufs=2))
        ep_pool = es.enter_context(tc.tile_pool(name="ep", bufs=2))
        ps_agg = es.enter_context(tc.tile_pool(name="ps_agg", bufs=4,
                                               space="PSUM"))
        ps_tr = es.enter_context(tc.tile_pool(name="ps_tr", bufs=2,
                                              space="PSUM"))
        ps_y = es.enter_context(tc.tile_pool(name="ps_y", bufs=2,
                                             space="PSUM"))

        # --- constants / resident inputs ---
        iota_row = const_pool.tile([128, 128], F32, tag="iota")
        nc.gpsimd.iota(iota_row[:], [[1, 128]], base=0, channel_multiplier=0,
                       allow_small_or_imprecise_dtypes=True)
        pidx = const_pool.tile([128, 1], F32, tag="pidx")
        nc.gpsimd.iota(pidx[:], [[1, 1]], base=0, channel_multiplier=1,
                       allow_small_or_imprecise_dtypes=True)
        ident = const_pool.tile([128, 128], BF16, tag="ident")
        nc.vector.tensor_scalar(ident[:], iota_row[:], pidx[:], None,
                                AluOpType.is_equal)
        epsb = const_pool.tile([128, 1], F32, tag="epsb")
        nc.vector.memset(epsb[:], LN_EPS)
        pwt_sb = const_pool.tile([128, 2, DIM], BF16, tag="pwt")
        nc.scalar.dma_start(pwt_sb[:], pwt_d[:])  # scalar: off W's ring

        # rel_embed (bf16), fully resident; chunked load so dst-tile 0's
        # matmuls can start before the whole 5 MB lands.  X rides the
        # scalar-engine HWDGE ring so it doesn't queue ahead of W's
        # sync-engine ring (per-engine FIFO).
        x_sb = const_pool.tile([128, S_TILES, XDIM], FP8, tag="x")
        bounds = X_BOUNDS
        for i in range(len(bounds) - 1):
            lo, hi = bounds[i], bounds[i + 1]
            nc.scalar.dma_start(x_sb[:, lo:hi, :],
                                x_d[:, lo * XDIM:hi * XDIM])

        def epilogue(t, agg_ps):
            # LN -> transpose -> @ proj_w.T -> residual.  PSUM col 256 holds
            # rowsum (X's 257th column trick); centered values are computed
            # straight from PSUM, so no agg copy sits on the critical chain.
            mean = ep_pool.tile([128, 1], F32, tag="mean")
            nc.scalar.mul(mean[:], agg_ps[:, 256:257], 1.0 / DIM)
            cent = ep_pool.tile([128, DIM], F32, tag="cent")
            nc.vector.tensor_scalar(cent[:], agg_ps[:, 0:DIM], mean[:], None,
                                    AluOpType.subtract)
            sq = ep_pool.tile([128, DIM], F32, tag="sq")
            sumsq = ep_pool.tile([128, 1], F32, tag="sumsq")
            nc.scalar.activation(sq[:], cent[:], AF.Square,
                                 accum_out=sumsq[:])
            std = ep_pool.tile([128, 1], F32, tag="std")
            nc.scalar.activation(std[:], sumsq[:], AF.Sqrt, bias=epsb[:],
                                 scale=1.0 / DIM)
            rstd = ep_pool.tile([128, 1], F32, tag="rstd")
            nc.vector.reciprocal(rstd[:], std[:])
            ln = ep_pool.tile([128, DIM], BF16, tag="ln")
            nc.vector.tensor_scalar(ln[:], cent[:], rstd[:], None,
                                    AluOpType.mult)

            y_ps = ps_y.tile([128, DIM], F32, tag="y")
            for k in range(2):
                tr_ps = ps_tr.tile([128, 128], BF16, tag="tr")
                nc.tensor.transpose(tr_ps[:], ln[:, k * 128:(k + 1) * 128],
                                    ident[:])
                lnT = ep_pool.tile([128, 128], BF16, tag="lnT")
                nc.scalar.copy(lnT[:], tr_ps[:])
                nc.tensor.matmul(y_ps[:], lnT[:], pwt_sb[:, k, :],
                                 start=(k == 0), stop=(k == 1))

            rel_t = ep_pool.tile([128, DIM], F32, tag="rel")
            nc.scalar.dma_start(rel_t[:], relsl_d[t * 128:(t + 1) * 128, :])
            delta = ep_pool.tile([128, DIM], F32, tag="delta")
            nc.vector.tensor_scalar(delta[:], y_ps[:], ALPHA, None,
                                    AluOpType.mult)
            out_t = ep_pool.tile([128, DIM], F32, tag="out")
            nc.vector.tensor_tensor(out_t[:], delta[:], rel_t[:],
                                    AluOpType.add)
            nc.sync.dma_start(out_d[t * 128:(t + 1) * 128, :], out_t[:])

        # software-pipelined: tile t's block MMs are emitted before tile
        # t-1's epilogue, so the PE (strict program order) never stalls on
        # the previous tile's LN chain -- it hides under the next MM run
        pending = []
        for t in range(N_TILES):
            wb = W_BOUNDS0 if t == 0 else W_BOUNDS
            w_t = wpool.tile([128, S_TILES, TILE], FP8, tag="w")
            for i in range(len(wb) - 1):
                lo, hi = wb[i], wb[i + 1]
                nc.sync.dma_start(w_t[:, lo:hi, :],
                                  w_d[t, :, lo * TILE:hi * TILE])

            agg_ps = ps_agg.tile([128, XDIM], F32, tag="agg")
            for s in range(S_TILES):
                nc.tensor.matmul(agg_ps[:], w_t[:, s, :], x_sb[:, s, :],
                                 start=(s == 0), stop=(s == S_TILES - 1))
            pending.append((t, agg_ps))
            if len(pending) > 2:  # depth-2: epilogue(t-2) after MMs(t)
                epilogue(*pending.pop(0))
        for p in pending:
            epilogue(*p)

    nc.compile()
    return nc


def _prep(rel_embed, rel_edge_index, rel_edge_weight, proj_w):
    """Host-side sharding/layout: scatter edges into dense per-(dst tile,
    src tile) weight blocks; lay out rel_embed for SBUF residency."""
    src = np.asarray(rel_edge_index[0], dtype=np.int64)
    dst = np.asarray(rel_edge_index[1], dtype=np.int64)
    w = np.asarray(rel_edge_weight, dtype=np.float32)
    rel = np.asarray(rel_embed, dtype=np.float32)
    pw = np.asarray(proj_w, dtype=np.float32)

    core = dst // NODES_PER_CORE
    drel = dst - core * NODES_PER_CORE
    t = drel // TILE
    d = drel % TILE
    s = src // TILE
    p = src % TILE
    # flat index inside one core's [N_TILES, S_TILES, 128, 128] block array
    flat = ((t * S_TILES + s) * TILE + p) * TILE + d
    blk_sz = N_TILES * S_TILES * TILE * TILE

    w_dev = np.empty((N_CORES, N_TILES, 128, S_TILES * TILE),
                     dtype=ml_dtypes.float8_e4m3)
    for c in range(N_CORES):
        m = core == c
        wc = np.bincount(flat[m], weights=w[m], minlength=blk_sz)
        wc = wc.reshape(N_TILES, S_TILES, TILE, TILE).astype(np.float32)
        # -> [t, p(src), s*128+d(dst)] so the SBUF tile is partition=src
        w_dev[c] = wc.transpose(0, 2, 1, 3).reshape(
            N_TILES, 128, S_TILES * TILE)

    rel16 = rel.astype(ml_dtypes.float8_e4m3)
    rel16_pad = np.zeros((S_TILES * TILE, XDIM), dtype=ml_dtypes.float8_e4m3)
    rel16_pad[:N_NODES, :DIM] = rel16
    rel16_pad[:N_NODES, DIM] = rel16.astype(np.float32).sum(axis=1).astype(
        ml_dtypes.float8_e4m3)
    x_dev = np.ascontiguousarray(
        rel16_pad.reshape(S_TILES, TILE, XDIM).transpose(1, 0, 2).reshape(
            128, S_TILES * XDIM))

    relslice = np.zeros((N_CORES, OUT_ROWS, DIM), dtype=np.float32)
    for c in range(N_CORES):
        relslice[c, :NODES_PER_CORE] = rel[c * NODES_PER_CORE:
                                           (c + 1) * NODES_PER_CORE]
    pwt = pw.T.astype(ml_dtypes.bfloat16)  # [f, o]
    pwt_dev = np.ascontiguousarray(
        pwt.reshape(2, 128, DIM).transpose(1, 0, 2).reshape(128, 2 * DIM))

    in_maps = []
    for c in range(N_CORES):
        in_maps.append({
            "x": x_dev,
            "wblk": w_dev[c],
            "relslice": relslice[c],
            "projwT": pwt_dev,
        })
    return in_maps


def kernel(rel_embed, rel_edge_index, rel_edge_weight, proj_w,
           _trace=False):
    in_maps = _prep(rel_embed, rel_edge_index, rel_edge_weight, proj_w)
    nc = _CACHE.get("nc")
    if nc is None:
        nc = _build()
        _CACHE["nc"] = nc
    res = run_bass_kernel_spmd(nc, in_maps, core_ids=list(range(N_CORES)),
                               trace=_trace)
    out = np.concatenate(
        [res.results[c]["out"][:NODES_PER_CORE] for c in range(N_CORES)],
        axis=0)
    if _trace:
        kernel.last_results = res
    return out.astype(np.float32)


# revision 40
# speedup vs baseline: 1.0305x; 1.0247x over previous
"""DualRelGCN message-passing kernel for 8 TRN2 NeuronCores.

Strategy (destination-sharded, collective-free, block-dense):
  - LayerNorm is invariant to positive per-row scaling, so LN(agg/denom) ==
    LN(agg): the denominator drops out of the computation entirely.
  - Shard edges by dst range: core c owns nodes [1250c, 1250(c+1)) and
    receives every edge whose dst falls there.  Each core computes its 1250
    output rows completely locally -> no collectives.
  - The weighted gather+segment_sum is expressed as a block matmul:
    agg[tile t] = sum_s W_ts.T @ X_s, where W_ts is the [128 src, 128 dst]
    dense block of the weighted adjacency (host-scattered from the edge
    list; ~5% nnz but dense matmul on PE beats any descriptor-generated
    gather path by a wide margin) and X_s is a [128, 256] tile of rel_embed
    (bf16, fully resident in SBUF).  PSUM accumulates over s in fp32.
  - Epilogue per dst tile: LN on DVE/ACT, PE transpose, y = ln @ proj_w.T,
    out = rel_embed + 0.1*y.
  - The device program is fully static: the edge distribution only changes
    tensor *contents*, never the instruction stream.
"""

import sys

for _p in ("/opt/trn_rl_repo",):
    if _p not in sys.path:
        sys.path.insert(0, _p)

from contextlib import ExitStack

import numpy as np
import ml_dtypes

import concourse.bacc as bacc
import concourse.mybir as mybir
from concourse.alu_op_type import AluOpType
from concourse.tile import TileContext
from concourse.bass_utils import run_bass_kernel_spmd

F32 = mybir.dt.float32
BF16 = mybir.dt.bfloat16
FP8 = mybir.dt.float8e4
AF = mybir.ActivationFunctionType

N_NODES = 10000
DIM = 256
N_CORES = 8
NODES_PER_CORE = N_NODES // N_CORES  # 1250
TILE = 128
N_TILES = -(-NODES_PER_CORE // TILE)  # 10 dst tiles per core
S_TILES = -(-N_NODES // TILE)  # 79 src tiles
OUT_ROWS = N_TILES * TILE  # 1280
ALPHA = 0.1
LN_EPS = 1e-5
# chunk boundaries (src-tile units): small leading chunks let the first
# matmuls start before the bulk of X/W lands
X_BOUNDS = [0, 20, 40, 60, 79]
XDIM = DIM
W_BOUNDS0 = [0, 20, 40, 60, 79]  # dst tile 0 (startup critical)
W_BOUNDS = [0, 20, 40, 60, 79]  # steady-state tiles

_CACHE: dict = {}


def _build():
    nc = bacc.Bacc("TRN2", target_bir_lowering=False, debug=False,
                   num_devices=N_CORES)

    x_d = nc.dram_tensor("x", [128, S_TILES * XDIM], FP8,
                        kind="ExternalInput")
    w_d = nc.dram_tensor("wblk", [N_TILES, 128, S_TILES * TILE], FP8,
                         kind="ExternalInput")
    relsl_d = nc.dram_tensor("relslice", [OUT_ROWS, DIM], F32,
                             kind="ExternalInput")
    pwt_d = nc.dram_tensor("projwT", [128, 2 * DIM], BF16,
                           kind="ExternalInput")
    out_d = nc.dram_tensor("out", [OUT_ROWS, DIM], F32, kind="ExternalOutput")

    with TileContext(nc) as tc, ExitStack() as es:
        const_pool = es.enter_context(tc.tile_pool(name="const", bufs=1))
        wpool = es.enter_context(tc.tile_pool(name="wblk", bufs=2))
        ep_pool = es.enter_context(tc.tile_pool(name="ep", bufs=2))
        ps_agg = es.enter_context(tc.tile_pool(name="ps_agg", bufs=3,
                                               space="PSUM"))
        ps_tr = es.enter_context(tc.tile_pool(name="ps_tr", bufs=2,
                                              space="PSUM"))
        ps_y = es.enter_context(tc.tile_pool(name="ps_y", bufs=2,
                                             space="PSUM"))

        # --- constants / resident inputs ---
        iota_row = const_pool.tile([128, 128], F32, tag="iota")
        nc.gpsimd.iota(iota_row[:], [[1, 128]], base=0, channel_multiplier=0,
                       allow_small_or_imprecise_dtypes=True)
        pidx = const_pool.tile([128, 1], F32, tag="pidx")
        nc.gpsimd.iota(pidx[:], [[1, 1]], base=0, channel_multiplier=1,
                       allow_small_or_imprecise_dtypes=True)
        ident = const_pool.tile([128, 128], BF16, tag="ident")
        nc.vector.tensor_scalar(ident[:], iota_row[:], pidx[:], None,
                                AluOpType.is_equal)
        epsb = const_pool.tile([128, 1], F32, tag="epsb")
        nc.vector.memset(epsb[:], LN_EPS)
        pwt_sb = const_pool.tile([128, 2, DIM], BF16, tag="pwt")
        nc.scalar.dma_start(pwt_sb[:], pwt_d[:])  # scalar: off W's ring

        # rel_embed (bf16), fully resident; chunked load so dst-tile 0's
        # matmuls can start before the whole 5 MB lands.  X rides the
        # scalar-engine HWDGE ring so it doesn't queue ahead of W's
        # sync-engine ring (per-engine FIFO).
        x_sb = const_pool.tile([128, S_TILES, XDIM], FP8, tag="x")
        bounds = X_BOUNDS
        for i in range(len(bounds) - 1):
            lo, hi = bounds[i], bounds[i + 1]
            nc.scalar.dma_start(x_sb[:, lo:hi, :],
                                x_d[:, lo * XDIM:hi * XDIM])

        def epilogue(t, agg_ps):
            # LN -> transpose -> @ proj_w.T -> residual
            agg = ep_pool.tile([128, DIM], F32, tag="agg_sb")
            rowsum = ep_pool.tile([128, 1], F32, tag="rowsum")
            nc.scalar.activation(agg[:], agg_ps[:], AF.Copy,
                                 accum_out=rowsum[:])
            mean = ep_pool.tile([128, 1], F32, tag="mean")
            nc.scalar.mul(mean[:], rowsum[:], 1.0 / DIM)
            cent = ep_pool.tile([128, DIM], F32, tag="cent")
            nc.vector.tensor_scalar(cent[:], agg[:], mean[:], None,
                                    AluOpType.subtract)
            sq = ep_pool.tile([128, DIM], F32, tag="sq")
            sumsq = ep_pool.tile([128, 1], F32, tag="sumsq")
            nc.scalar.activation(sq[:], cent[:], AF.Square,
                                 accum_out=sumsq[:])
            std = ep_pool.tile([128, 1], F32, tag="std")
            nc.scalar.activation(std[:], sumsq[:], AF.Sqrt, bias=epsb[:],
                                 scale=1.0 / DIM)
            rstd = ep_pool.tile([128, 1], F32, tag="rstd")
            nc.vector.reciprocal(rstd[:], std[:])
            ln = ep_pool.tile([128, DIM], BF16, tag="ln")
            nc.vector.tensor_scalar(ln[:], cent[:], rstd[:], None,
                                    AluOpType.mult)

            y_ps = ps_y.tile([128, DIM], F32, tag="y")
            for k in range(2):
                tr_ps = ps_tr.tile([128, 128], BF16, tag="tr")
                nc.tensor.transpose(tr_ps[:], ln[:, k * 128:(k + 1) * 128],
                                    ident[:])
                lnT = ep_pool.tile([128, 128], BF16, tag="lnT")
                nc.scalar.copy(lnT[:], tr_ps[:])
                nc.tensor.matmul(y_ps[:], lnT[:], pwt_sb[:, k, :],
                                 start=(k == 0), stop=(k == 1))

            rel_t = ep_pool.tile([128, DIM], F32, tag="rel")
            nc.scalar.dma_start(rel_t[:], relsl_d[t * 128:(t + 1) * 128, :])
            delta = ep_pool.tile([128, DIM], F32, tag="delta")
            nc.vector.tensor_scalar(delta[:], y_ps[:], ALPHA, None,
                                    AluOpType.mult)
            out_t = ep_pool.tile([128, DIM], F32, tag="out")
            nc.vector.tensor_tensor(out_t[:], delta[:], rel_t[:],
                                    AluOpType.add)
            nc.sync.dma_start(out_d[t * 128:(t + 1) * 128, :], out_t[:])

        # software-pipelined: tile t's block MMs are emitted before tile
        # t-1's epilogue, so the PE (strict program order) never stalls on
        # the previous tile's LN chain -- it hides under the next MM run
        pending = []
        for t in range(N_TILES):
            wb = W_BOUNDS0 if t == 0 else W_BOUNDS
            w_t = wpool.tile([128, S_TILES, TILE], FP8, tag="w")
            for i in range(len(wb) - 1):
                lo, hi = wb[i], wb[i + 1]
                nc.sync.dma_start(w_t[:, lo:hi, :],
                                  w_d[t, :, lo * TILE:hi * TILE])

            agg_ps = ps_agg.tile([128, XDIM], F32, tag="agg")
            for s in range(S_TILES):
                nc.tensor.matmul(agg_ps[:], w_t[:, s, :], x_sb[:, s, :],
                                 start=(s == 0), stop=(s == S_TILES - 1))
            pending.append((t, agg_ps))
            if len(pending) > 1:
                epilogue(*pending.pop(0))
        for p in pending:
            epilogue(*p)

    nc.compile()
    return nc


def _prep(rel_embed, rel_edge_index, rel_edge_weight, proj_w):
    """Host-side sharding/layout: scatter edges into dense per-(dst tile,
    src tile) weight blocks; lay out rel_embed for SBUF residency."""
    src = np.asarray(rel_edge_index[0], dtype=np.int64)
    dst = np.asarray(rel_edge_index[1], dtype=np.int64)
    w = np.asarray(rel_edge_weight, dtype=np.float32)
    rel = np.asarray(rel_embed, dtype=np.float32)
    pw = np.asarray(proj_w, dtype=np.float32)

    core = dst // NODES_PER_CORE
    drel = dst - core * NODES_PER_CORE
    t = drel // TILE
    d = drel % TILE
    s = src // TILE
    p = src % TILE
    # flat index inside one core's [N_TILES, S_TILES, 128, 128] block array
    flat = ((t * S_TILES + s) * TILE + p) * TILE + d
    blk_sz = N_TILES * S_TILES * TILE * TILE

    w_dev = np.empty((N_CORES, N_TILES, 128, S_TILES * TILE),
                     dtype=ml_dtypes.float8_e4m3)
    for c in range(N_CORES):
        m = core == c
        wc = np.bincount(flat[m], weights=w[m], minlength=blk_sz)
        wc = wc.reshape(N_TILES, S_TILES, TILE, TILE).astype(np.float32)
        # -> [t, p(src), s*128+d(dst)] so the SBUF tile is partition=src
        w_dev[c] = wc.transpose(0, 2, 1, 3).reshape(
            N_TILES, 128, S_TILES * TILE)

    rel16 = rel.astype(ml_dtypes.float8_e4m3)
    rel16_pad = np.zeros((S_TILES * TILE, XDIM), dtype=ml_dtypes.float8_e4m3)
    rel16_pad[:N_NODES, :DIM] = rel16
    x_dev = np.ascontiguousarray(
        rel16_pad.reshape(S_TILES, TILE, XDIM).transpose(1, 0, 2).reshape(
            128, S_TILES * XDIM))

    relslice = np.zeros((N_CORES, OUT_ROWS, DIM), dtype=np.float32)
    for c in range(N_CORES):
        relslice[c, :NODES_PER_CORE] = rel[c * NODES_PER_CORE:
                                           (c + 1) * NODES_PER_CORE]
    pwt = pw.T.astype(ml_dtypes.bfloat16)  # [f, o]
    pwt_dev = np.ascontiguousarray(
        pwt.reshape(2, 128, DIM).transpose(1, 0, 2).reshape(128, 2 * DIM))

    in_maps = []
    for c in range(N_CORES):
        in_maps.append({
            "x": x_dev,
            "wblk": w_dev[c],
            "relslice": relslice[c],
            "projwT": pwt_dev,
        })
    return in_maps


def kernel(rel_embed, rel_edge_index, rel_edge_weight, proj_w,
           _trace=False):
    in_maps = _prep(rel_embed, rel_edge_index, rel_edge_weight, proj_w)
    nc = _CACHE.get("nc")
    if nc is None:
        nc = _build()
        _CACHE["nc"] = nc
    res = run_bass_kernel_spmd(nc, in_maps, core_ids=list(range(N_CORES)),
                               trace=_trace)
    out = np.concatenate(
        [res.results[c]["out"][:NODES_PER_CORE] for c in range(N_CORES)],
        axis=0)
    if _trace:
        kernel.last_results = res
    return out.astype(np.float32)
